# revision 44
# baseline (speedup 1.0000x reference)
# kernel.py — Trainium2 Bass kernel for nn_Net_17188459119113 (quantized CNN).
#
# Pipeline (per reference.py):
#   xq = quant4(x); wq = quant4(conv_w)
#   y  = conv2d(xq, wq, VALID) + b; relu; maxpool 4x4/4; flatten
#   fq = quant4(flat); out = fq @ quant4(fc_w).T + fc_b
#
# Strategy: pure data-parallel over 8 NeuronCores (batch 8192 -> 1024/core).
# Everything on-device runs in the integer domain (quantized values are small
# exact integers in f16/f32); affine scales are applied late.
#
# v2 design (vs the v1 baseline, which was ~411us in the cost-model sim):
#  - x arrives host-transposed as [112=(bq4,h28), 7168=(bb8,b32,w28)] f32 so
#    quantization (fp32 magic-round) runs on 112 partitions, and the banded
#    conv input x3 [84=(dj3,h28), (b,w)] is built by 12 big-run SBUF->SBUF
#    DMAs per half (the dj shift reads a contiguous run with a 2-element
#    garbage tail that the matmul rhs never addresses).
#  - conv = banded matmul, K=84=(dj,h), M=128=(16oc x 8i), N=(16b x 24j);
#    3 stationaries (c = i-octet) reused across all chunks -> few LDWEIGHTS.
#  - W-pool (max over j%4) happens at PSUM drain time, split across two
#    engines: some PSUM groups drain via DVE tensor_reduce directly, the
#    rest via ACT f32->f16 convert (jm-major relayout) + DVE/GPSIMD
#    tensor_tensor max tree at 2x f16 throughput.  PSUM volume (9.4M f32
#    per core) crossing DVE/ACT once is the hard roofline of this net.
#  - H-pool (max over i%4): partition-extract DMAs (i%4 == m partitions of
#    the stage-1 output) into 4 tiles + a 3-op f16 TT-max tree.
#  - relu/bias/FC-quant are deferred past the (monotone) max pools and the
#    AllReduce(max) that produces the global flat scale; the tail is
#    relu-scale (ACT) + magic-round (DVE) + 12 accumulating FC matmuls.
# Output returned as [10, 1024] per core, transposed/concatenated on host.

import numpy as np

P = 128
B_CORE = 1024  # images per core
NCORES = 8
MAGIC = float(np.float32(12582912.0))  # 1.5 * 2**23: fp32 RNE rounding trick
NEG_BIG = -3.0e38

# --- tuning knobs (env-overridable for sweeps) ------------------------------
import os as _os
# Drain route per PSUM group, cycled over the 48 groups: 'd' = DVE
# tensor_reduce direct, 'a' = ACT convert + TT tree.
ROUTE_PATTERN = _os.environ.get("K_ROUTE", "dddddd" + "aad" * 14)
# Engine for the jm TT trees of act-routed groups, cycled: 'v'=vector,
# 'g'=gpsimd.
TREE_PATTERN = _os.environ.get("K_TREE", "v")
# Engine for the stage-2 (H-pool) TT trees, cycled per half: 'v' or 'g'.
S2_PATTERN = _os.environ.get("K_S2", "v")

_NC = None  # cached compiled Bass module (input-independent)


def _f32(v):
    return np.float32(v)


def _host_quant_scale(t):
    # mirrors reference _quant scale computation in fp32 arithmetic
    n = _f32(7.0)
    m = np.max(np.abs(t.astype(np.float32))).astype(np.float32)
    return _f32(_f32(m / n) + _f32(1e-8))


def _build_nc(sim=False):
    import concourse.bass as bass
    import concourse.mybir as mybir
    from concourse import bacc, bass_isa
    from concourse.tile import TileContext

    f32 = mybir.dt.float32
    f16 = mybir.dt.float16
    AF = mybir.ActivationFunctionType
    OP = mybir.AluOpType

    nc = bacc.Bacc(None, num_devices=1 if sim else NCORES)

    # x: [112=(h28,bq4), 7168=(bb8,b32,w28)] + 8 pad cols (shift tail reads)
    XCOLS = 7168 + 8
    x_in = nc.declare_dram_parameter("x", [112, XCOLS], f32, isOutput=False)
    # wts: [96, 448] = w3 banded [84, 384] (rows 84:96 zero) ++ fw [96, 60+4pad]
    wts_in = nc.declare_dram_parameter("wts", [96, 448], f16, isOutput=False)
    # meta: [128, 6] = scal(inv_sx, s_xw, s_fw, 0) ++ cb96 col4 ++ fb col5
    meta_in = nc.declare_dram_parameter("meta", [P, 6], f32, isOutput=False)
    out_ext = nc.declare_dram_parameter("out", [10, B_CORE], f32, isOutput=True)

    cc_in = nc.dram_tensor("cc_in", [1, 512], f32)
    cc_out = nc.dram_tensor("cc_out", [1, 512], f32,
                            addr_space="Local" if sim else "Shared")

    route_i = [0]
    tree_i = [0]

    def next_route():
        r = ROUTE_PATTERN[route_i[0] % len(ROUTE_PATTERN)]
        route_i[0] += 1
        return r

    def next_tree():
        t = TREE_PATTERN[tree_i[0] % len(TREE_PATTERN)]
        tree_i[0] += 1
        return t

    with TileContext(nc, num_cores=1 if sim else NCORES) as tc:
        with tc.tile_pool(name="const", bufs=1) as cpool:
            wts = cpool.tile([96, 448], f16)
            meta = cpool.tile([P, 6], f32)
            magic = cpool.tile([P, 1], f32)
            lm128 = cpool.tile([P, 1], f32)
            lm2 = cpool.tile([96, 4], f32)
            nc.sync.dma_start(out=wts[:, :], in_=wts_in[:, :])
            nc.sync.dma_start(out=meta[:, :], in_=meta_in[:, :])
            w3sb = wts[0:84, 0:384]
            fwsb = wts[0:96, 384:444]
            scal = meta[:, 0:4]
            cb96 = meta[0:96, 4:5]
            fb16 = meta[0:16, 5:6]
            nmagic = cpool.tile([P, 1], f32)
            eps = cpool.tile([P, 1], f32)
            nc.vector.memset(eps[:, :], float(np.float32(1e-8)))
            nc.vector.memset(magic[:, :], MAGIC)
            nc.vector.memset(nmagic[:, :], -MAGIC)
            nc.vector.memset(lm128[:, :], 0.0)  # relu floor for global max

            # persistent across the whole pipeline
            with tc.tile_pool(name="persist", bufs=1) as ppool:
                xq = ppool.tile([112, XCOLS], f16)
                # 6240 = (1024+16)*6: 16 pad image-slots so the [ch4,b32]
                # drain view of the last group stays in-bounds
                s1 = [ppool.tile([P, 6240], f16, name=f"s1_{i}")
                      for i in range(3)]  # per c
                flat = ppool.tile([96, 6144], f16)

                _emit_pipeline(
                    nc, tc, mybir, AF, OP, sim,
                    x_in, w3sb, scal, magic, nmagic, xq, s1, flat, lm2)

                # ---- tail: global scale, FC quant, FC matmuls ----
                _emit_tail(
                    nc, tc, mybir, AF, OP, sim, bass_isa,
                    cc_in, cc_out, out_ext,
                    scal, magic, eps, cb96, fb16, fwsb, lm128, lm2, flat)

    nc.finalize()
    return nc


def _emit_pipeline(nc, tc, mybir, AF, OP, sim,
                   x_in, w3sb, scal, magic, nmagic, xq, s1, flat, lm2):
    """Quant + conv + W-pool drains + H-pool, pipelined over 2 halves."""
    f32 = mybir.dt.float32
    f16 = mybir.dt.float16

    route_idx = [0]
    tree_idx = [0]
    s2_idx = [0]

    with (
        tc.tile_pool(name="xf", bufs=4) as xfpool,
        tc.tile_pool(name="x3", bufs=1) as x3pool,
        tc.tile_pool(name="jms", bufs=2) as jmpool,
        tc.tile_pool(name="tt", bufs=2) as ttpool,
        tc.tile_pool(name="s2", bufs=2) as s2pool,
        tc.tile_pool(name="tm", bufs=1) as tmpool,
        tc.tile_pool(name="ps", bufs=2, space="PSUM") as pspool,
    ):
        tmt = [tmpool.tile([96, 6144], f16, name=f"tm_{i}") for i in range(3)]

        # ---- 1) quantize all of x: 8 col-chunks, non-overlapping writes
        # (global chunk k writes [896k + (4 if k else 0), 896k+900); the +4
        #  skip avoids re-writing the previous chunk's shift-overlap cols).
        # Pass 1 (fma + magic) on ACT; pass 2 (sub magic -> f16) on DVE,
        # which is otherwise idle during the ramp.  SP/HWDGE order
        # interleaves the x3 builds between the loads so the first conv
        # matmul can start at ~8us.
        def emit_quant(k):
            ws = 896 * k + (4 if k else 0)
            we = 896 * k + 900
            w = we - ws
            xf = xfpool.tile([112, 900], f32, name="xf")
            nc.sync.dma_start(out=xf[:, 0:w], in_=x_in[:, ws:we])
            tq = xfpool.tile([112, 900], f32, tag="tq", name="tq")
            nc.scalar.activation(
                out=tq[:, 0:w], in_=xf[:, 0:w], func=AF.Identity,
                bias=magic[0:112, 0:1], scale=scal[0:112, 0:1])
            if k < 2:
                # ramp: DVE is idle, shorten the ACT critical chain
                nc.vector.tensor_scalar(
                    out=xq[:, ws:we], in0=tq[:, 0:w], scalar1=MAGIC,
                    scalar2=None, op0=OP.subtract)
            else:
                nc.scalar.activation(
                    out=xq[:, ws:we], in_=tq[:, 0:w], func=AF.Identity,
                    bias=nmagic[0:112, 0:1], scale=1.0)

        # ---- 2) build x3 for both halves:
        # x3h[half] = [84=(dj,h), (bq4, bbh4, b32, w28)]; per (half, part, dj)
        # ONE DMA: xq partitions are (h, bq) h-major, so the 112-partition
        # source folds into 28 dst partitions x 4 bq free-blocks with
        # matching iteration order (1792-elem contiguous runs at offset dj;
        # the 2-elem garbage tail is never read by the matmul rhs).
        x3hs = [x3pool.tile([84, 14336], f16, name=f"x3h_{h}")
                for h in range(2)]

        def emit_x3(half, part):
            hc0 = half * 3584
            x3h = x3hs[half]
            for dj in range(3):
                src = xq[0:112,
                         hc0 + 1792 * part + dj:
                         hc0 + 1792 * part + dj + 1792]
                dst = x3h[28 * dj:28 * (dj + 1), :].rearrange(
                    "p (bq pr f) -> p bq pr f", bq=4, pr=2)[:, :, part, :]
                nc.sync.dma_start(out=dst, in_=src)

        for k in range(4):
            emit_quant(k)
        emit_x3(0, 0)  # slots its transfers before the later loads
        for k in range(4, 8):
            emit_quant(k)
        emit_x3(0, 1)
        emit_x3(1, 0)
        emit_x3(1, 1)

        # ---- 3) conv + W-pool drains + extracts ----
        pend = {"jms": None, "n": 0}

        def emit_group(half, bbh, c, g):
            x3v = x3hs[half][:, :].rearrange(
                "p (bq bbh b w) -> p bq bbh b w", bq=4, bbh=4, w=28)
            s1v = s1[c][:, :].rearrange("p (b jw) -> p b jw", jw=6)
            ps = pspool.tile([P, 2048], f32, tag="ps", name="ps")
            for ch in range(4):  # chunk = (bq=ch, bhalf=g)
                rhs = x3v[:, ch, bbh, g * 16:(g + 1) * 16, 0:24]
                nc.tensor.matmul(
                    out=ps[:, ch * 512:ch * 512 + 384],
                    lhsT=w3sb[:, c * 128:(c + 1) * 128],
                    rhs=rhs, start=True, stop=True)
            # PSUM view [p, ch4, b16, jw6, jm4]
            pin = ps[:, :].rearrange(
                "p (ch x) -> p ch x", ch=4)[:, :, 0:384].rearrange(
                "p ch (b jw jm) -> p ch b jw jm", b=16, jm=4)
            bg0 = (half * 4 + bbh) * 128 + g * 16
            # S1 dest view [p, ch4, b16, jw6] at b = bg0 + ch*32 + b
            sout = s1v[:, bg0:bg0 + 128, :].rearrange(
                "p (ch b) jw -> p ch b jw", b=32)[:, :, 0:16, :]
            r = ROUTE_PATTERN[route_idx[0] % len(ROUTE_PATTERN)]
            route_idx[0] += 1
            if r == "d":
                nc.vector.tensor_reduce(
                    out=sout, in_=pin, axis=mybir.AxisListType.X, op=OP.max)
                return
            jms = jmpool.tile([P, 1536], f16, name="jms")
            jview = jms[:, :].rearrange(
                "p (jm ch b jw) -> p jm ch b jw", jm=4, ch=4, b=16)
            nc.scalar.activation(
                out=jview.rearrange("p jm ch b jw -> p ch b jw jm"),
                in_=pin, func=AF.Identity)
            ta = ttpool.tile([P, 384], f16, tag="ta", name="ta")
            tb = ttpool.tile([P, 384], f16, tag="tb", name="tb")
            nc.vector.tensor_tensor(
                ta[:, :], jms[:, 0:384], jms[:, 384:768], OP.max)
            nc.vector.tensor_tensor(
                tb[:, :], jms[:, 768:1152], jms[:, 1152:1536], OP.max)
            t4 = "p (ch b jw) -> p ch b jw"
            nc.vector.tensor_tensor(
                sout, ta[:, :].rearrange(t4, ch=4, b=16),
                tb[:, :].rearrange(t4, ch=4, b=16), OP.max)

        def emit_extracts(half, c2):
            # flat/tmt partition layout q = oc*6 + iwb*3 + c (c minor):
            # src partitions m::4 iterate (oc, iwb)-lex, dst partitions c::3
            # iterate the same; one single-stride DMA per (c, m).
            # m=3 goes straight into flat (saves one T tile).
            f0 = half * 3072
            for m in range(4):
                src = s1[c2][m::4, f0:f0 + 3072]
                dstt = flat if m == 3 else tmt[m]
                dst = dstt[c2::3, f0:f0 + 3072]
                nc.sync.dma_start(out=dst, in_=src)

        def emit_trees(half):
            f0 = half * 3072
            for k in range(2):
                g0 = f0 + k * 1536
                s2e = S2_PATTERN[s2_idx[0] % len(S2_PATTERN)]
                s2_idx[0] += 1
                eng2 = nc.vector if s2e == "v" else nc.gpsimd
                ua = s2pool.tile([96, 1536], f16, tag="ua", name="ua")
                ub = s2pool.tile([96, 1536], f16, tag="ub", name="ub")
                eng2.tensor_tensor(
                    ua[:, :], tmt[0][:, g0:g0 + 1536],
                    tmt[1][:, g0:g0 + 1536], OP.max)
                eng2.tensor_tensor(
                    ub[:, :], tmt[2][:, g0:g0 + 1536],
                    flat[:, g0:g0 + 1536], OP.max)
                eng2.tensor_tensor(
                    flat[:, g0:g0 + 1536], ua[:, :], ub[:, :], OP.max)
                nc.vector.tensor_reduce(
                    out=lm2[:, 2 * half + k:2 * half + k + 1],
                    in_=flat[:, g0:g0 + 1536],
                    axis=mybir.AxisListType.X, op=OP.max)

        # half 0: bbh-outer (conv can start right after the first quant
        # chunks); extracts+trees at half end, trees on Pool so they
        # overlap half 1's conv.
        for bbh in range(4):
            for c in range(3):
                for g in range(2):
                    emit_group(0, bbh, c, g)
        for c in range(3):
            emit_extracts(0, c)
        emit_trees(0)
        # half 1: c-outer so each c's extracts (SP/HWDGE chain) overlap the
        # remaining conv; trees on DVE at the end.
        for c in range(3):
            for bbh in range(4):
                for g in range(2):
                    emit_group(1, bbh, c, g)
            emit_extracts(1, c)
        emit_trees(1)


def _emit_tail(nc, tc, mybir, AF, OP, sim, bass_isa,
               cc_in, cc_out, out_ext,
               scal, magic, eps, cb96, fb16, fwsb, lm128, lm2, flat):
    f32 = mybir.dt.float32
    f16 = mybir.dt.float16

    with (
        tc.tile_pool(name="sm2", bufs=2) as smpool,
        tc.tile_pool(name="tt2", bufs=4) as tpool,
        tc.tile_pool(name="psfc", bufs=2, space="PSUM") as pfcpool,
        tc.tile_pool(name="outp", bufs=2) as outpool,
        tc.tile_pool(name="fqp", bufs=1) as fqpool,
    ):
        fq = fqpool.tile([96, 6144], f16)
        # ---- t1' = relu(s_xw*flat + cb): s_f-independent, so it runs
        # during the collective wait (half 0 even earlier) ----
        t1p = tpool.tile([96, 6144], f32, tag="t1p")
        for k in range(4):
            f0 = k * 1536
            nc.scalar.activation(
                out=t1p[:, f0:f0 + 1536], in_=flat[:, f0:f0 + 1536],
                func=AF.Relu, bias=cb96[:, 0:1], scale=scal[0:96, 1:2])

        # ---- global flat max -> s_f ----
        lmI = smpool.tile([96, 1], f32, tag="lmI")
        nc.vector.tensor_reduce(
            out=lmI[:, :], in_=lm2[:, :], axis=mybir.AxisListType.X, op=OP.max)
        # real pre-relu max per partition: relu(s_xw * int_max + conv_b)
        nc.scalar.activation(
            out=lm128[0:96, 0:1], in_=lmI[:, :], func=AF.Relu,
            bias=cb96[:, 0:1], scale=scal[0:96, 1:2])
        lmr = smpool.tile([P, 1], f32, tag="lmr")
        nc.gpsimd.partition_all_reduce(
            lmr[:, :], lm128[:, :], 128, bass_isa.ReduceOp.max)
        # only slot 0 of the 512-slot payload is meaningful; the AllReduce
        # is elementwise max, so the other slots' garbage never reaches it
        nc.sync.dma_start(out=cc_in[0:1, 0:1], in_=lmr[0:1, 0:1])
        if sim:
            nc.sync.dma_start(out=cc_out[0:1, 0:1], in_=cc_in[0:1, 0:1])
        else:
            nc.gpsimd.collective_compute(
                "AllReduce", OP.max,
                replica_groups=[list(range(NCORES))],
                ins=[cc_in[:, :]], outs=[cc_out[:, :]])
        gmb = smpool.tile([P, 1], f32, tag="gmb")
        nc.sync.dma_start(
            out=gmb[:, :], in_=cc_out[0:1, 0:1].to_broadcast((P, 1)))
        # s_f = gmax/7 + 1e-8 (gmax >= 0); invsf = 1/s_f (both on ACT);
        # sprod = s_f * s_fw (DVE)
        sf = smpool.tile([P, 1], f32, tag="sf")
        nc.scalar.activation(
            out=sf[:, :], in_=gmb[:, :], func=AF.Identity,
            bias=eps[:, 0:1], scale=float(np.float32(1.0) / np.float32(7.0)))
        invsf = smpool.tile([P, 1], f32, tag="invsf")
        nc.vector.reciprocal(out=invsf[:, :], in_=sf[:, :])
        sprod = smpool.tile([P, 1], f32, tag="sprod")
        nc.vector.tensor_scalar(
            out=sprod[:, :], in0=sf[:, :], scalar1=scal[:, 2:3],
            scalar2=None, op0=OP.mult)

        # ---- FC quant: t2 = t1'*invsf + MAGIC; fq = t2 - MAGIC (f16);
        # interleaved with the FC matmuls per b-half ----
        fqv = fq[:, :].rearrange("p (b jw) -> p b jw", jw=6)
        for bh in range(2):
            for sub in range(2):
                f0 = bh * 3072 + sub * 1536
                t2 = tpool.tile([96, 1536], f32, tag="t2")
                nc.vector.tensor_scalar(
                    out=t2[:, :], in0=t1p[:, f0:f0 + 1536],
                    scalar1=invsf[0:96, 0:1], scalar2=MAGIC,
                    op0=OP.mult, op1=OP.add)
                nc.vector.tensor_scalar(
                    out=fq[:, f0:f0 + 1536], in0=t2[:, :], scalar1=MAGIC,
                    scalar2=None, op0=OP.subtract)
            psfc = pfcpool.tile([16, 512], f32)
            for jw in range(6):
                nc.tensor.matmul(
                    out=psfc[0:10, :],
                    lhsT=fwsb[:, jw * 10:(jw + 1) * 10],
                    rhs=fqv[:, bh * 512:(bh + 1) * 512, jw],
                    start=(jw == 0), stop=(jw == 5))
            osb = outpool.tile([16, 512], f32)
            nc.scalar.activation(
                out=osb[0:10, :], in_=psfc[0:10, :], func=AF.Identity,
                bias=fb16[0:10, 0:1], scale=sprod[0:10, 0:1])
            nc.sync.dma_start(
                out=out_ext[:, bh * 512:(bh + 1) * 512], in_=osb[0:10, :])


def _host_constants(x, conv_w, conv_b, fc_w, fc_b):
    s_x = _host_quant_scale(x)
    s_w = _host_quant_scale(conv_w)
    s_fw = _host_quant_scale(fc_w)
    kw = np.round(conv_w.astype(np.float32) / s_w).astype(np.float32)
    kfw = np.round(fc_w.astype(np.float32) / s_fw).astype(np.float32)

    # banded conv weights: w3[(dj,h), c*128 + oc*8 + isub] = kw[oc, h-i, dj],
    # i = 8c + isub
    w3 = np.zeros((84, 384), np.float32)
    for dj in range(3):
        for c in range(3):
            for isub in range(8):
                i = 8 * c + isub
                for di in range(3):
                    h = i + di
                    if h < 28:
                        for oc in range(16):
                            w3[28 * dj + h, c * 128 + oc * 8 + isub] = \
                                kw[oc, 0, di, dj]

    # FC weights in the flat partition layout q = oc*6 + iwb*3 + c
    # (iw = 2c + iwb): fw[q, jw*10 + cls] = kfw[cls, oc*36 + iw*6 + jw]
    fw = np.zeros((96, 60), np.float32)
    for oc in range(16):
        for iwb in range(2):
            for c in range(3):
                q = oc * 6 + iwb * 3 + c
                iw = 2 * c + iwb
                for jw in range(6):
                    k = oc * 36 + iw * 6 + jw
                    fw[q, jw * 10:(jw + 1) * 10] = kfw[:, k]

    wts = np.zeros((96, 448), np.float32)
    wts[0:84, 0:384] = w3
    wts[:, 384:444] = fw

    meta = np.zeros((P, 6), np.float32)
    inv_sx = _f32(_f32(1.0) / s_x)
    s_xw = _f32(s_x * s_w)
    meta[:, 0] = inv_sx
    meta[:, 1] = s_xw
    meta[:, 2] = s_fw
    meta[0:96, 4] = np.repeat(conv_b.astype(np.float32), 6)  # cb[q] = b[q//6]
    meta[0:10, 5] = fc_b.astype(np.float32)

    return {"wts": wts.astype(np.float16), "meta": meta}


def _host_x_shard(x, core):
    # [1024,1,28,28] -> [112=(h28,bq4), 7168=(bb8,b32,w28)] + 8 pad cols
    xs = x[core * B_CORE:(core + 1) * B_CORE].reshape(8, 4, 32, 28, 28)
    xt = xs.transpose(3, 1, 0, 2, 4).reshape(112, 7168)
    out = np.zeros((112, 7168 + 8), np.float32)
    out[:, :7168] = xt
    return out


def _get_nc():
    global _NC
    if _NC is None:
        _NC = _build_nc()
    return _NC


def kernel(x, conv_w, conv_b, fc_w, fc_b, _trace=False):
    from concourse.bass_utils import run_bass_kernel_spmd

    x = np.asarray(x, np.float32)
    consts = _host_constants(
        x, np.asarray(conv_w, np.float32), np.asarray(conv_b, np.float32),
        np.asarray(fc_w, np.float32), np.asarray(fc_b, np.float32))

    nc = _get_nc()
    in_maps = []
    for c in range(NCORES):
        m = {"x": _host_x_shard(x, c)}
        m.update(consts)
        in_maps.append(m)

    res = run_bass_kernel_spmd(nc, in_maps, list(range(NCORES)), trace=_trace)
    out = np.concatenate([r["out"].T for r in res.results], axis=0)
    if _trace:
        kernel._last_results = res
    return np.ascontiguousarray(out.astype(np.float32))


# revision 52
# speedup vs baseline: 1.0433x; 1.0433x over previous
# kernel.py — Trainium2 Bass kernel for nn_Net_17188459119113 (quantized CNN).
#
# Pipeline (per reference.py):
#   xq = quant4(x); wq = quant4(conv_w)
#   y  = conv2d(xq, wq, VALID) + b; relu; maxpool 4x4/4; flatten
#   fq = quant4(flat); out = fq @ quant4(fc_w).T + fc_b
#
# Strategy: pure data-parallel over 8 NeuronCores (batch 8192 -> 1024/core).
# Everything on-device runs in the integer domain (quantized values are small
# exact integers in f16/f32); affine scales are applied late.
#
# v2 design (vs the v1 baseline, which was ~411us in the cost-model sim):
#  - x arrives host-transposed as [112=(bq4,h28), 7168=(bb8,b32,w28)] f32 so
#    quantization (fp32 magic-round) runs on 112 partitions, and the banded
#    conv input x3 [84=(dj3,h28), (b,w)] is built by 12 big-run SBUF->SBUF
#    DMAs per half (the dj shift reads a contiguous run with a 2-element
#    garbage tail that the matmul rhs never addresses).
#  - conv = banded matmul, K=84=(dj,h), M=128=(16oc x 8i), N=(16b x 24j);
#    3 stationaries (c = i-octet) reused across all chunks -> few LDWEIGHTS.
#  - W-pool (max over j%4) happens at PSUM drain time, split across two
#    engines: some PSUM groups drain via DVE tensor_reduce directly, the
#    rest via ACT f32->f16 convert (jm-major relayout) + DVE/GPSIMD
#    tensor_tensor max tree at 2x f16 throughput.  PSUM volume (9.4M f32
#    per core) crossing DVE/ACT once is the hard roofline of this net.
#  - H-pool (max over i%4): partition-extract DMAs (i%4 == m partitions of
#    the stage-1 output) into 4 tiles + a 3-op f16 TT-max tree.
#  - relu/bias/FC-quant are deferred past the (monotone) max pools and the
#    AllReduce(max) that produces the global flat scale; the tail is
#    relu-scale (ACT) + magic-round (DVE) + 12 accumulating FC matmuls.
# Output returned as [10, 1024] per core, transposed/concatenated on host.

import numpy as np

P = 128
B_CORE = 1024  # images per core
NCORES = 8
MAGIC = float(np.float32(12582912.0))  # 1.5 * 2**23: fp32 RNE rounding trick
NEG_BIG = -3.0e38

# --- tuning knobs (env-overridable for sweeps) ------------------------------
import os as _os
# Drain route per PSUM group, cycled over the 48 groups: 'd' = DVE
# tensor_reduce direct, 'a' = ACT convert + TT tree.
ROUTE_PATTERN = _os.environ.get("K_ROUTE", "dddddd" + "aad" * 14)
# Engine for the jm TT trees of act-routed groups, cycled: 'v'=vector,
# 'g'=gpsimd.
TREE_PATTERN = _os.environ.get("K_TREE", "v")
# Engine for the stage-2 (H-pool) TT trees, cycled per half: 'v' or 'g'.
S2_PATTERN = _os.environ.get("K_S2", "v")
# Global-max source: "s1" = reduce pre-H-pool s1 tiles (earlier collective,
# +DVE work); "flat" = reduce the pooled flat (less DVE, later collective).
LM_MODE = _os.environ.get("K_LM", "hyb")

_NC = None  # cached compiled Bass module (input-independent)


def _f32(v):
    return np.float32(v)


def _host_quant_scale(t):
    # mirrors reference _quant scale computation in fp32 arithmetic
    n = _f32(7.0)
    m = np.max(np.abs(t.astype(np.float32))).astype(np.float32)
    return _f32(_f32(m / n) + _f32(1e-8))


def _build_nc(sim=False):
    import concourse.bass as bass
    import concourse.mybir as mybir
    from concourse import bacc, bass_isa
    from concourse.tile import TileContext

    f32 = mybir.dt.float32
    f16 = mybir.dt.float16
    AF = mybir.ActivationFunctionType
    OP = mybir.AluOpType

    nc = bacc.Bacc(None, num_devices=1 if sim else NCORES)

    # x: [112=(h28,bq4), 7168=(bb8,b32,w28)] + 8 pad cols (shift tail reads)
    XCOLS = 7168 + 8
    x_in = nc.declare_dram_parameter("x", [112, XCOLS], f32, isOutput=False)
    # wts: [96, 448] = w3 banded [84, 384] (rows 84:96 zero) ++ fw [96, 60+4pad]
    wts_in = nc.declare_dram_parameter("wts", [96, 448], f16, isOutput=False)
    # meta: [128, 6] = scal(inv_sx, s_xw, s_fw, 0) ++ cb96 col4 ++ fb col5
    meta_in = nc.declare_dram_parameter("meta", [P, 6], f32, isOutput=False)
    out_ext = nc.declare_dram_parameter("out", [10, B_CORE], f32, isOutput=True)

    cc_in = nc.dram_tensor("cc_in", [1, 512], f32)
    cc_out = nc.dram_tensor("cc_out", [1, 512], f32,
                            addr_space="Local" if sim else "Shared")
    cc_mid = nc.dram_tensor("cc_mid", [1, 16384], f32) if sim else None

    route_i = [0]
    tree_i = [0]

    def next_route():
        r = ROUTE_PATTERN[route_i[0] % len(ROUTE_PATTERN)]
        route_i[0] += 1
        return r

    def next_tree():
        t = TREE_PATTERN[tree_i[0] % len(TREE_PATTERN)]
        tree_i[0] += 1
        return t

    with TileContext(nc, num_cores=1 if sim else NCORES) as tc:
        with tc.tile_pool(name="const", bufs=1) as cpool:
            wts = cpool.tile([96, 448], f16)
            meta = cpool.tile([P, 6], f32)
            magic = cpool.tile([P, 1], f32)
            lm128 = cpool.tile([P, 1], f32)
            lm2 = cpool.tile([P, 6], f32)
            lmF = cpool.tile([96, 6], f32)
            nc.sync.dma_start(out=wts[:, :], in_=wts_in[:, :])
            nc.sync.dma_start(out=meta[:, :], in_=meta_in[:, :])
            w3sb = wts[0:84, 0:384]
            fwsb = wts[0:96, 384:444]
            scal = meta[:, 0:4]
            cb96 = meta[0:96, 4:5]
            fb16 = meta[0:16, 5:6]
            nmagic = cpool.tile([P, 1], f32)
            eps = cpool.tile([P, 1], f32)
            nc.vector.memset(eps[:, :], float(np.float32(1e-8)))
            nc.vector.memset(magic[:, :], MAGIC)
            nc.vector.memset(nmagic[:, :], -MAGIC)
            nc.vector.memset(lm128[:, :], 0.0)  # relu floor for global max
            nc.vector.memset(lm2[:, :], NEG_BIG)
            nc.vector.memset(lmF[:, :], NEG_BIG)

            # persistent across the whole pipeline
            with tc.tile_pool(name="persist", bufs=1) as ppool:
                xq = ppool.tile([112, XCOLS], f16)
                # 6240 = (1024+16)*6: 16 pad image-slots so the [ch4,b32]
                # drain view of the last group stays in-bounds
                s1 = [ppool.tile([P, 6240], f16, name=f"s1_{i}")
                      for i in range(3)]  # per c
                flat = ppool.tile([96, 6144], f16)

                _emit_pipeline(
                    nc, tc, mybir, AF, OP, sim,
                    x_in, w3sb, scal, magic, nmagic, xq, s1, flat, lm2, lmF)

                # ---- tail: global scale, FC quant, FC matmuls ----
                _emit_tail(
                    nc, tc, mybir, AF, OP, sim, bass_isa,
                    cc_in, cc_out, cc_mid, out_ext,
                    scal, magic, nmagic, eps, cb96, fb16, fwsb, lm128, lm2,
                    lmF, flat)

    nc.finalize()
    return nc


def _emit_pipeline(nc, tc, mybir, AF, OP, sim,
                   x_in, w3sb, scal, magic, nmagic, xq, s1, flat, lm2, lmF):
    """Quant + conv + W-pool drains + H-pool, pipelined over 2 halves."""
    f32 = mybir.dt.float32
    f16 = mybir.dt.float16

    route_idx = [0]
    tree_idx = [0]
    s2_idx = [0]

    with (
        tc.tile_pool(name="xf", bufs=4) as xfpool,
        tc.tile_pool(name="x3", bufs=1) as x3pool,
        tc.tile_pool(name="jms", bufs=2) as jmpool,
        tc.tile_pool(name="tt", bufs=2) as ttpool,
        tc.tile_pool(name="s2", bufs=2) as s2pool,
        tc.tile_pool(name="tm", bufs=1) as tmpool,
        tc.tile_pool(name="ps", bufs=2, space="PSUM") as pspool,
    ):
        tmt = [tmpool.tile([96, 6144], f16, name=f"tm_{i}") for i in range(3)]

        # ---- 1) quantize all of x: 8 col-chunks, non-overlapping writes
        # (global chunk k writes [896k + (4 if k else 0), 896k+900); the +4
        #  skip avoids re-writing the previous chunk's shift-overlap cols).
        # Pass 1 (fma + magic) on ACT; pass 2 (sub magic -> f16) on DVE,
        # which is otherwise idle during the ramp.  SP/HWDGE order
        # interleaves the x3 builds between the loads so the first conv
        # matmul can start at ~8us.
        def emit_quant(k):
            ws = 896 * k + (4 if k else 0)
            we = 896 * k + 900
            w = we - ws
            xf = xfpool.tile([112, 900], f32, name="xf")
            nc.sync.dma_start(out=xf[:, 0:w], in_=x_in[:, ws:we])
            tq = xfpool.tile([112, 900], f32, tag="tq", name="tq")
            nc.scalar.activation(
                out=tq[:, 0:w], in_=xf[:, 0:w], func=AF.Identity,
                bias=magic[0:112, 0:1], scale=scal[0:112, 0:1])
            if k < 2:
                # ramp: DVE is idle, shorten the ACT critical chain
                nc.vector.tensor_scalar(
                    out=xq[:, ws:we], in0=tq[:, 0:w], scalar1=MAGIC,
                    scalar2=None, op0=OP.subtract)
            else:
                nc.scalar.activation(
                    out=xq[:, ws:we], in_=tq[:, 0:w], func=AF.Identity,
                    bias=nmagic[0:112, 0:1], scale=1.0)

        # ---- 2) build x3 for both halves:
        # x3h[half] = [84=(dj,h), (bq4, bbh4, b32, w28)]; per (half, part, dj)
        # ONE DMA: xq partitions are (h, bq) h-major, so the 112-partition
        # source folds into 28 dst partitions x 4 bq free-blocks with
        # matching iteration order (1792-elem contiguous runs at offset dj;
        # the 2-elem garbage tail is never read by the matmul rhs).
        x3hs = [x3pool.tile([84, 14336], f16, name=f"x3h_{h}")
                for h in range(2)]

        def emit_x3(half, part):
            hc0 = half * 3584
            x3h = x3hs[half]
            for dj in range(3):
                src = xq[0:112,
                         hc0 + 1792 * part + dj:
                         hc0 + 1792 * part + dj + 1792]
                dst = x3h[28 * dj:28 * (dj + 1), :].rearrange(
                    "p (bq pr f) -> p bq pr f", bq=4, pr=2)[:, :, part, :]
                nc.sync.dma_start(out=dst, in_=src)

        for k in range(2):
            emit_quant(k)
        emit_x3(0, 0)  # needs only chunks 0-1; transfers before later loads
        for k in range(2, 4):
            emit_quant(k)
        for k in range(4, 8):
            emit_quant(k)
        emit_x3(0, 1)
        emit_x3(1, 0)
        emit_x3(1, 1)

        # ---- 3) conv + W-pool drains + extracts ----
        pend = {"jms": None, "n": 0}

        def emit_group(half, bbh, c, g):
            x3v = x3hs[half][:, :].rearrange(
                "p (bq bbh b w) -> p bq bbh b w", bq=4, bbh=4, w=28)
            s1v = s1[c][:, :].rearrange("p (b jw) -> p b jw", jw=6)
            ps = pspool.tile([P, 2048], f32, tag="ps", name="ps")
            for ch in range(4):  # chunk = (bq=ch, bhalf=g)
                rhs = x3v[:, ch, bbh, g * 16:(g + 1) * 16, 0:24]
                nc.tensor.matmul(
                    out=ps[:, ch * 512:ch * 512 + 384],
                    lhsT=w3sb[:, c * 128:(c + 1) * 128],
                    rhs=rhs, start=True, stop=True)
            # PSUM view [p, ch4, b16, jw6, jm4]
            pin = ps[:, :].rearrange(
                "p (ch x) -> p ch x", ch=4)[:, :, 0:384].rearrange(
                "p ch (b jw jm) -> p ch b jw jm", b=16, jm=4)
            bg0 = (half * 4 + bbh) * 128 + g * 16
            # S1 dest view [p, ch4, b16, jw6] at b = bg0 + ch*32 + b
            sout = s1v[:, bg0:bg0 + 128, :].rearrange(
                "p (ch b) jw -> p ch b jw", b=32)[:, :, 0:16, :]
            r = ROUTE_PATTERN[route_idx[0] % len(ROUTE_PATTERN)]
            route_idx[0] += 1
            if r == "d":
                nc.vector.tensor_reduce(
                    out=sout, in_=pin, axis=mybir.AxisListType.X, op=OP.max)
                return
            jms = jmpool.tile([P, 1536], f16, name="jms")
            jview = jms[:, :].rearrange(
                "p (jm ch b jw) -> p jm ch b jw", jm=4, ch=4, b=16)
            nc.scalar.activation(
                out=jview.rearrange("p jm ch b jw -> p ch b jw jm"),
                in_=pin, func=AF.Identity)
            ta = ttpool.tile([P, 384], f16, tag="ta", name="ta")
            tb = ttpool.tile([P, 384], f16, tag="tb", name="tb")
            nc.vector.tensor_tensor(
                ta[:, :], jms[:, 0:384], jms[:, 384:768], OP.max)
            nc.vector.tensor_tensor(
                tb[:, :], jms[:, 768:1152], jms[:, 1152:1536], OP.max)
            t4 = "p (ch b jw) -> p ch b jw"
            nc.vector.tensor_tensor(
                sout, ta[:, :].rearrange(t4, ch=4, b=16),
                tb[:, :].rearrange(t4, ch=4, b=16), OP.max)

        def emit_lm(half, c2):
            # global-max shortcut: max(flat) == max(s1) (H-pool discards
            # nothing for a max), so the collective input needs no extracts.
            # hyb: s1-based only for half 1 (the collective critical path);
            # half 0 uses the cheaper flat-based reduces.
            if LM_MODE == "flat" or (LM_MODE == "hyb" and half == 0):
                return
            nc.vector.tensor_reduce(
                out=lm2[:, 3 * half + c2:3 * half + c2 + 1],
                in_=s1[c2][:, half * 3072:half * 3072 + 3072],
                axis=mybir.AxisListType.X, op=OP.max)

        def emit_extracts(half, c2):
            # flat/tmt partition layout q = oc*6 + iwb*3 + c (c minor):
            # src partitions m::4 iterate (oc, iwb)-lex, dst partitions c::3
            # iterate the same; one single-stride DMA per (c, m).
            # m=3 goes straight into flat (saves one T tile).
            f0 = half * 3072
            for m in range(4):
                src = s1[c2][m::4, f0:f0 + 3072]
                dstt = flat if m == 3 else tmt[m]
                dst = dstt[c2::3, f0:f0 + 3072]
                nc.sync.dma_start(out=dst, in_=src)

        def emit_trees(half):
            f0 = half * 3072
            for k in range(2):
                g0 = f0 + k * 1536
                s2e = S2_PATTERN[s2_idx[0] % len(S2_PATTERN)]
                s2_idx[0] += 1
                eng2 = nc.vector if s2e == "v" else nc.gpsimd
                ua = s2pool.tile([96, 1536], f16, tag="ua", name="ua")
                ub = s2pool.tile([96, 1536], f16, tag="ub", name="ub")
                eng2.tensor_tensor(
                    ua[:, :], tmt[0][:, g0:g0 + 1536],
                    tmt[1][:, g0:g0 + 1536], OP.max)
                eng2.tensor_tensor(
                    ub[:, :], tmt[2][:, g0:g0 + 1536],
                    flat[:, g0:g0 + 1536], OP.max)
                eng2.tensor_tensor(
                    flat[:, g0:g0 + 1536], ua[:, :], ub[:, :], OP.max)
                if LM_MODE == "flat" or (LM_MODE == "hyb" and half == 0):
                    nc.vector.tensor_reduce(
                        out=lmF[:, 3 * half + k:3 * half + k + 1],
                        in_=flat[:, g0:g0 + 1536],
                        axis=mybir.AxisListType.X, op=OP.max)

        # half 0: bbh-outer (conv can start right after the first quant
        # chunks); extracts+trees at half end, trees on Pool so they
        # overlap half 1's conv.
        for bbh in range(4):
            for c in range(3):
                for g in range(2):
                    emit_group(0, bbh, c, g)
        for c in range(3):
            emit_lm(0, c)
            emit_extracts(0, c)
        emit_trees(0)
        # half 1: c-outer so each c's extracts (SP/HWDGE chain) overlap the
        # remaining conv; trees on DVE at the end.
        for c in range(3):
            for bbh in range(4):
                for g in range(2):
                    emit_group(1, bbh, c, g)
            emit_lm(1, c)
            emit_extracts(1, c)
        emit_trees(1)


def _emit_tail(nc, tc, mybir, AF, OP, sim, bass_isa,
               cc_in, cc_out, cc_mid, out_ext,
               scal, magic, nmagic, eps, cb96, fb16, fwsb, lm128, lm2, lmF,
               flat):
    f32 = mybir.dt.float32
    f16 = mybir.dt.float16

    with (
        tc.tile_pool(name="sm2", bufs=2) as smpool,
        tc.tile_pool(name="tt2", bufs=4) as tpool,
        tc.tile_pool(name="psfc", bufs=2, space="PSUM") as pfcpool,
        tc.tile_pool(name="outp", bufs=2) as outpool,
        tc.tile_pool(name="fqp", bufs=1) as fqpool,
    ):
        fq = fqpool.tile([96, 6144], f16)
        # ---- t1' = relu(s_xw*flat + cb): s_f-independent, so it runs
        # during the collective wait (half 0 even earlier) ----
        t1p = tpool.tile([96, 6144], f32, tag="t1p")
        for k in range(4):
            f0 = k * 1536
            nc.scalar.activation(
                out=t1p[:, f0:f0 + 1536], in_=flat[:, f0:f0 + 1536],
                func=AF.Relu, bias=cb96[:, 0:1], scale=scal[0:96, 1:2])

        # ---- global flat max -> s_f ----
        # s1-meaning columns (partitions (oc,isub), bias conv_b[p//8]) and
        # flat-meaning columns (partitions q=(oc,iwb,c), bias conv_b[q//6])
        # are reduced+biased separately, then maxed into lm128.
        if LM_MODE in ("s1", "hyb"):
            lmI = smpool.tile([P, 1], f32, tag="lmI")
            nc.vector.tensor_reduce(
                out=lmI[:, :], in_=lm2[:, :],
                axis=mybir.AxisListType.X, op=OP.max)
            nc.scalar.activation(
                out=lm128[:, 0:1], in_=lmI[:, :], func=AF.Relu,
                bias=scal[:, 3:4], scale=scal[:, 1:2])
        if LM_MODE in ("flat", "hyb"):
            lmJ = smpool.tile([96, 1], f32, tag="lmJ")
            nc.vector.tensor_reduce(
                out=lmJ[:, :], in_=lmF[:, :],
                axis=mybir.AxisListType.X, op=OP.max)
            lmK = smpool.tile([96, 1], f32, tag="lmK")
            nc.scalar.activation(
                out=lmK[:, :], in_=lmJ[:, :], func=AF.Relu,
                bias=cb96[:, 0:1], scale=scal[0:96, 1:2])
            if LM_MODE == "flat":
                nc.vector.tensor_copy(out=lm128[0:96, 0:1], in_=lmK[:, :])
            else:
                nc.vector.tensor_tensor(
                    lm128[0:96, 0:1], lm128[0:96, 0:1], lmK[:, :], OP.max)
        lmr = smpool.tile([P, 1], f32, tag="lmr")
        nc.gpsimd.partition_all_reduce(
            lmr[:, :], lm128[:, :], 128, bass_isa.ReduceOp.max)
        # only slot 0 of the 512-slot payload is meaningful; the AllReduce
        # is elementwise max, so the other slots' garbage never reaches it
        nc.sync.dma_start(out=cc_in[0:1, 0:1], in_=lmr[0:1, 0:1])
        if sim:
            # stand-in for the real AllReduce latency (~15us const in the
            # cost model): a serial chain of DRAM round-trips
            nc.sync.dma_start(out=cc_mid[0:1, 0:512], in_=cc_in[0:1, :])
            for h in range(6):
                nc.sync.dma_start(
                    out=cc_mid[0:1, (h + 1) * 512:(h + 2) * 512],
                    in_=cc_mid[0:1, h * 512:(h + 1) * 512])
            nc.sync.dma_start(out=cc_out[0:1, 0:1], in_=cc_mid[0:1, 6*512:6*512+1])
        else:
            nc.gpsimd.collective_compute(
                "AllReduce", OP.max,
                replica_groups=[list(range(NCORES))],
                ins=[cc_in[:, :]], outs=[cc_out[:, :]])
        gmb = smpool.tile([P, 1], f32, tag="gmb")
        nc.sync.dma_start(
            out=gmb[:, :], in_=cc_out[0:1, 0:1].to_broadcast((P, 1)))
        # s_f = gmax/7 + 1e-8 (gmax >= 0); invsf = 1/s_f (both on ACT);
        # sprod = s_f * s_fw (DVE)
        sf = smpool.tile([P, 1], f32, tag="sf")
        nc.scalar.activation(
            out=sf[:, :], in_=gmb[:, :], func=AF.Identity,
            bias=eps[:, 0:1], scale=float(np.float32(1.0) / np.float32(7.0)))
        invsf = smpool.tile([P, 1], f32, tag="invsf")
        nc.vector.reciprocal(out=invsf[:, :], in_=sf[:, :])
        sprod = smpool.tile([P, 1], f32, tag="sprod")
        nc.vector.tensor_scalar(
            out=sprod[:, :], in0=sf[:, :], scalar1=scal[:, 2:3],
            scalar2=None, op0=OP.mult)

        # ---- FC quant: t2 = t1'*invsf + MAGIC; fq = t2 - MAGIC (f16);
        # interleaved with the FC matmuls per b-half ----
        fqv = fq[:, :].rearrange("p (b jw) -> p b jw", jw=6)
        for bh in range(2):
            for sub in range(2):
                f0 = bh * 3072 + sub * 1536
                t2 = tpool.tile([96, 1536], f32, tag="t2")
                if sub == 0:
                    nc.vector.tensor_scalar(
                        out=t2[:, :], in0=t1p[:, f0:f0 + 1536],
                        scalar1=invsf[0:96, 0:1], scalar2=MAGIC,
                        op0=OP.mult, op1=OP.add)
                    nc.vector.tensor_scalar(
                        out=fq[:, f0:f0 + 1536], in0=t2[:, :], scalar1=MAGIC,
                        scalar2=None, op0=OP.subtract)
                else:
                    nc.scalar.activation(
                        out=t2[:, :], in_=t1p[:, f0:f0 + 1536],
                        func=AF.Identity, bias=magic[0:96, 0:1],
                        scale=invsf[0:96, 0:1])
                    nc.scalar.activation(
                        out=fq[:, f0:f0 + 1536], in_=t2[:, :],
                        func=AF.Identity, bias=nmagic[0:96, 0:1], scale=1.0)
            psfc = pfcpool.tile([16, 512], f32)
            for jw in range(6):
                nc.tensor.matmul(
                    out=psfc[0:10, :],
                    lhsT=fwsb[:, jw * 10:(jw + 1) * 10],
                    rhs=fqv[:, bh * 512:(bh + 1) * 512, jw],
                    start=(jw == 0), stop=(jw == 5))
            osb = outpool.tile([16, 512], f32)
            nc.scalar.activation(
                out=osb[0:10, :], in_=psfc[0:10, :], func=AF.Identity,
                bias=fb16[0:10, 0:1], scale=sprod[0:10, 0:1])
            nc.sync.dma_start(
                out=out_ext[:, bh * 512:(bh + 1) * 512], in_=osb[0:10, :])


def _host_constants(x, conv_w, conv_b, fc_w, fc_b):
    s_x = _host_quant_scale(x)
    s_w = _host_quant_scale(conv_w)
    s_fw = _host_quant_scale(fc_w)
    kw = np.round(conv_w.astype(np.float32) / s_w).astype(np.float32)
    kfw = np.round(fc_w.astype(np.float32) / s_fw).astype(np.float32)

    # banded conv weights: w3[(dj,h), c*128 + oc*8 + isub] = kw[oc, h-i, dj],
    # i = 8c + isub
    w3 = np.zeros((84, 384), np.float32)
    for dj in range(3):
        for c in range(3):
            for isub in range(8):
                i = 8 * c + isub
                for di in range(3):
                    h = i + di
                    if h < 28:
                        for oc in range(16):
                            w3[28 * dj + h, c * 128 + oc * 8 + isub] = \
                                kw[oc, 0, di, dj]

    # FC weights in the flat partition layout q = oc*6 + iwb*3 + c
    # (iw = 2c + iwb): fw[q, jw*10 + cls] = kfw[cls, oc*36 + iw*6 + jw]
    fw = np.zeros((96, 60), np.float32)
    for oc in range(16):
        for iwb in range(2):
            for c in range(3):
                q = oc * 6 + iwb * 3 + c
                iw = 2 * c + iwb
                for jw in range(6):
                    k = oc * 36 + iw * 6 + jw
                    fw[q, jw * 10:(jw + 1) * 10] = kfw[:, k]

    wts = np.zeros((96, 448), np.float32)
    wts[0:84, 0:384] = w3
    wts[:, 384:444] = fw

    meta = np.zeros((P, 6), np.float32)
    inv_sx = _f32(_f32(1.0) / s_x)
    s_xw = _f32(s_x * s_w)
    meta[:, 0] = inv_sx
    meta[:, 1] = s_xw
    meta[:, 2] = s_fw
    meta[0:96, 4] = np.repeat(conv_b.astype(np.float32), 6)  # cb[q] = b[q//6]
    meta[0:10, 5] = fc_b.astype(np.float32)
    meta[:, 3] = np.repeat(conv_b.astype(np.float32), 8)  # lm bias b[p//8]

    return {"wts": wts.astype(np.float16), "meta": meta}


def _host_x_shard(x, core):
    # [1024,1,28,28] -> [112=(h28,bq4), 7168=(bb8,b32,w28)] + 8 pad cols
    xs = x[core * B_CORE:(core + 1) * B_CORE].reshape(8, 4, 32, 28, 28)
    xt = xs.transpose(3, 1, 0, 2, 4).reshape(112, 7168)
    out = np.zeros((112, 7168 + 8), np.float32)
    out[:, :7168] = xt
    return out


def _get_nc():
    global _NC
    if _NC is None:
        _NC = _build_nc()
    return _NC


def kernel(x, conv_w, conv_b, fc_w, fc_b, _trace=False):
    from concourse.bass_utils import run_bass_kernel_spmd

    x = np.asarray(x, np.float32)
    consts = _host_constants(
        x, np.asarray(conv_w, np.float32), np.asarray(conv_b, np.float32),
        np.asarray(fc_w, np.float32), np.asarray(fc_b, np.float32))

    nc = _get_nc()
    in_maps = []
    for c in range(NCORES):
        m = {"x": _host_x_shard(x, c)}
        m.update(consts)
        in_maps.append(m)

    res = run_bass_kernel_spmd(nc, in_maps, list(range(NCORES)), trace=_trace)
    out = np.concatenate([r["out"].T for r in res.results], axis=0)
    if _trace:
        kernel._last_results = res
    return np.ascontiguousarray(out.astype(np.float32))
